# revision 11
# baseline (speedup 1.0000x reference)
"""Bahdanau-attention kernel for Trainium2, 8-core data-parallel over batch.

Problem: context = softmax(w2 . tanh(enc @ W1_enc + hid @ W1_hid + b1)) @ enc
  B=32, S=2048, D=1024.  Each of the 8 cores handles 4 batch elements.

Device-side strategy (per core, per batch b, per 512-wide seq chunk):
  - the heavy enc @ W1_enc matmul runs in fp8-e4m3 with DoubleRow perf
    mode: both operands quantized to e4m3 (W1 pre-scaled by 64 into the
    e4m3 normal range), two 128-deep k-tiles contracted per PE pass.
    The 1/64 de-scale rides the ACT tanh's per-op scale for free.
  - chunks are processed in PAIRS sharing each stationary weight load:
    LDWEIGHTS costs ~470ns when weights change between matmuls but is
    fully hidden when two consecutive matmuls share them (measured).
  - tanh+bias via ACT: h = tanh(hp/64 + z[m]), output bf16.
  - e-scores: e_row[2, 512] += w2[m]^T @ h[m] in bf16 on the PE,
    emitted TWO m-steps behind the hp matmuls (m=6,7 spill into the
    next pair's first two slots) so the PE never waits on ACT.
  - softmax post-work runs entirely off the PE: p_row = exp(e_row) on
    ACT (fused accum_out -> normalizer partial; no max subtraction:
    |e| <= sum|w2| ~ 26, safe fp32), p broadcast to 128 partitions on
    GPSIMD, context partials on DVE via fused affine_mul_reduce over a
    separate bf16 copy of encT (one op per k-group does multiply by p
    and the 512-wide reduction).
  - z = hid @ W1_hid + b1 computed as hid-stationary matmuls (tiny
    4-col weight loads) giving zT[4, 1024], then PE-transposed per
    m-chunk via a 4x4 identity and bias-added on DVE.  Emitted inside
    pair 0's slot 1 so the PE never waits on the W1_hid DMA.
  - context[b] = ctx * (1/Z), 1/Z partition-broadcast on GPSIMD.

Quantization error budget (validated vs the fp32 reference on the
harness inputs, and on hardware): fp8 enc/W1 + bf16 h/w2/hid/W1h ==>
rel_err ~1.22e-2 against the 2e-2 gate.
"""

import numpy as np
import ml_dtypes
from contextlib import ExitStack

import concourse.bacc as bacc
import concourse.tile as tile
from concourse import mybir
from concourse.bass_utils import run_bass_kernel_spmd

AFT = mybir.ActivationFunctionType
ALU = mybir.AluOpType
F32 = mybir.dt.float32
BF16 = mybir.dt.bfloat16
F8 = mybir.dt.float8e4
DR = mybir.MatmulPerfMode.DoubleRow

B, S, D = 32, 2048, 1024
NCORES = 8
BL = B // NCORES          # 4 batch elements per core
P = 128
KC = D // P               # 8 contraction / output chunks
KP = KC // 2              # 4 DoubleRow k-pairs
S_SUB = 512               # seq chunk processed per inner iteration
NSS = S // S_SUB          # 4
NPAIR = NSS // 2          # chunk pairs per batch
NQ = BL * NPAIR           # total pairs
W1_SCALE = 64.0           # fp8 pre-scale on W1_enc (power of two)


def declare_io(nc, input_kind="ExternalInput"):
    t = {}
    t["encT8"] = nc.dram_tensor("encT8", [BL, D, S], F8, kind=input_kind).ap()
    t["encTb"] = nc.dram_tensor("encTb", [BL, D, S], BF16,
                                kind=input_kind).ap()
    t["hidT"] = nc.dram_tensor("hidT", [D, BL], BF16, kind=input_kind).ap()
    t["w1e8"] = nc.dram_tensor("w1e8", [D, D], F8, kind=input_kind).ap()
    t["w1h"] = nc.dram_tensor("w1h", [D, D], BF16, kind=input_kind).ap()
    t["b1"] = nc.dram_tensor("b1", [D, 1], F32, kind=input_kind).ap()
    t["w2"] = nc.dram_tensor("w2", [D, 2], BF16, kind=input_kind).ap()
    t["ctx_out"] = nc.dram_tensor("ctx_out", [BL, D], F32,
                                  kind="ExternalOutput").ap()
    return t


def _body(ctx, tc, t):
    nc = tc.nc
    id4 = nc.inline_tensor(np.eye(BL, dtype=np.float32), name="id4").ap()

    const = ctx.enter_context(tc.tile_pool(name="const", bufs=1))
    wpool = ctx.enter_context(tc.tile_pool(name="wpool", bufs=1))
    epool = ctx.enter_context(tc.tile_pool(name="epool", bufs=4))
    bpool = ctx.enter_context(tc.tile_pool(name="bpool", bufs=6))
    hpool = ctx.enter_context(tc.tile_pool(name="hpool", bufs=8))
    spool = ctx.enter_context(tc.tile_pool(name="spool", bufs=2))
    pbpool = ctx.enter_context(tc.tile_pool(name="pbpool", bufs=2))
    s2pool = ctx.enter_context(tc.tile_pool(name="s2pool", bufs=2))
    cpool = ctx.enter_context(tc.tile_pool(name="cpool", bufs=2 * NSS + 2))
    # PSUM budget (8 banks): hp 4 + (e_ps | ztp) 2 + zT halves 2
    ppa = ctx.enter_context(tc.tile_pool(name="ppa", bufs=4, space="PSUM"))
    ppe = ctx.enter_context(tc.tile_pool(name="ppe", bufs=2, space="PSUM"))
    pzt = ctx.enter_context(tc.tile_pool(name="pzt", bufs=2, space="PSUM"))

    encT8, encTb = t["encT8"], t["encTb"]

    # --- phase 0 DMAs, one instruction per tensor (HWDGE costs ~625ns
    # of serialized issue per DMA instruction, so batch via 3D access
    # patterns).  Order = first-use order on the PE.
    w1e_all = wpool.tile([P, KC, D], F8, name="w1e_all")
    nc.sync.dma_start(w1e_all[:],
                      t["w1e8"].rearrange("(k p) m -> p k m", p=P))

    et8_tiles, etb_tiles = {}, {}
    encT8r = [encT8[b].rearrange("(k p) s -> p k s", p=P)
              for b in range(BL)]
    encTbr = [encTb[b].rearrange("(k p) s -> p k s", p=P)
              for b in range(BL)]

    def dma_pair8(q):
        """fp8 et for both chunks of pair q (feeds the hp matmuls)."""
        b, sp = divmod(q, NPAIR)
        for ss in (2 * sp, 2 * sp + 1):
            e8 = epool.tile([P, KC, S_SUB], F8, name="et8", tag="et8")
            nc.sync.dma_start(
                e8[:], encT8r[b][:, :, ss * S_SUB:(ss + 1) * S_SUB])
            et8_tiles[(b, ss)] = e8

    def dma_pairb(q):
        """bf16 et for both chunks of pair q (feeds the DVE context)."""
        b, sp = divmod(q, NPAIR)
        for ss in (2 * sp, 2 * sp + 1):
            eb = bpool.tile([P, KC, S_SUB], BF16, name="etb", tag="etb")
            nc.sync.dma_start(
                eb[:], encTbr[b][:, :, ss * S_SUB:(ss + 1) * S_SUB])
            etb_tiles[(b, ss)] = eb

    dma_pair8(0)

    hid_all = const.tile([P, KC, BL], BF16, name="hid_all")
    nc.sync.dma_start(hid_all[:],
                      t["hidT"].rearrange("(k p) b -> p k b", p=P))
    w1h_all = wpool.tile([P, KC, D], BF16, name="w1h_all")
    nc.sync.dma_start(w1h_all[:],
                      t["w1h"].rearrange("(k p) m -> p k m", p=P))
    b1_all = const.tile([P, KC], F32, name="b1_all")
    nc.sync.dma_start(b1_all[:],
                      t["b1"].rearrange("(k p) o -> p (k o)", p=P))
    w2_all = const.tile([P, KC, 2], BF16, name="w2_all")
    nc.sync.dma_start(w2_all[:],
                      t["w2"].rearrange("(k p) c -> p k c", p=P))
    id4_t = const.tile([BL, BL], F32, name="id4_t")
    nc.sync.dma_start(id4_t[:], id4)
    dma_pairb(0)

    z_sb = []            # [P, BL] per m, filled by emit_z_block

    def emit_z_block():
        """z = hid @ W1_hid + b1 via hid-stationary matmuls (zT[4, e])
        + per-m PE transpose.  Sits in pair 0's slot 1 so the W1_hid
        DMA has landed by the time the PE gets here."""
        zt0 = pzt.tile([BL, S_SUB], F32, name="zt0", tag="zt")
        zt1 = pzt.tile([BL, S_SUB], F32, name="zt1", tag="zt")
        for k in range(KC):
            nc.tensor.matmul(zt0[:], lhsT=hid_all[:, k, :],
                             rhs=w1h_all[:, k, 0:S_SUB],
                             start=(k == 0), stop=(k == KC - 1))
            nc.tensor.matmul(zt1[:], lhsT=hid_all[:, k, :],
                             rhs=w1h_all[:, k, S_SUB:D],
                             start=(k == 0), stop=(k == KC - 1))
        zts = const.tile([BL, D], F32, name="zts")
        nc.vector.tensor_copy(zts[:, 0:S_SUB], zt0[:])
        nc.vector.tensor_copy(zts[:, S_SUB:D], zt1[:])
        for m in range(KC):
            ztp = ppe.tile([P, BL], F32, name="ztp", tag="ppe_t")
            nc.tensor.transpose(ztp[:], zts[:, m * P:(m + 1) * P], id4_t[:])
            zt = const.tile([P, BL], F32, name=f"z_{m}")
            nc.vector.tensor_scalar_add(zt[:], ztp[:], b1_all[:, m:m + 1])
            z_sb.append(zt)

    state = {}           # per-batch: z_parts tile + per-chunk cred tiles

    def emit_exp(pend):
        """exp + gpsimd partition-broadcast for a finished chunk; returns
        the broadcast tile for the amr stage."""
        pb, pss, e_ps = pend
        st = state[pb]
        p_row = spool.tile([1, S_SUB], F32, name="p_row", tag="p_row")
        nc.scalar.activation(p_row[:], e_ps[0:1, :], AFT.Exp,
                             accum_out=st["z_parts"][0:1, pss:pss + 1])
        p_bc = pbpool.tile([P, S_SUB], F32, name="p_bc", tag="p_bc")
        nc.gpsimd.partition_broadcast(p_bc[:], p_row[:])
        return p_bc

    def emit_ctx(pend, p_bc):
        """DVE context partials for a finished chunk (+finalize)."""
        pb, pss, _ = pend
        st = state[pb]
        etb = etb_tiles.pop((pb, pss))
        cred = cpool.tile([P, KC], F32, name="cred", tag="cred")
        for k in range(KC):
            scr = s2pool.tile([P, S_SUB], BF16, name="scr", tag="scr")
            nc.vector.affine_mul_reduce(
                out=scr[:], accum_out=cred[:, k:k + 1],
                in0=etb[:, k, :], in1=p_bc[:], scale=1.0, bias=0.0)
        st["creds"].append(cred)
        if pss == NSS - 1:
            emit_finalize(pb)

    def emit_finalize(pb):
        """context[pb] = (sum of chunk contributions) / Z, then store."""
        st = state.pop(pb)
        zsum = spool.tile([1, 1], F32, name="zsum", tag="zsum")
        nc.vector.tensor_reduce(zsum[:], st["z_parts"][:],
                                axis=mybir.AxisListType.X, op=ALU.add)
        zr = spool.tile([1, 1], F32, name="zr", tag="zr")
        nc.vector.reciprocal(zr[:], zsum[:])
        zr_bc = spool.tile([P, 1], F32, name="zr_bc", tag="zr_bc")
        nc.gpsimd.partition_broadcast(zr_bc[:], zr[:])
        creds = st["creds"]
        ctx_fin = cpool.tile([P, KC], F32, name="ctx_fin", tag="ctx_fin")
        nc.vector.tensor_tensor(ctx_fin[:], creds[0][:], creds[1][:], ALU.add)
        nc.vector.tensor_tensor(ctx_fin[:], ctx_fin[:], creds[2][:], ALU.add)
        nc.vector.tensor_tensor(ctx_fin[:], ctx_fin[:], creds[3][:], ALU.add)
        ctx_sc = cpool.tile([P, KC], F32, name="ctx_sc", tag="ctx_sc")
        nc.vector.tensor_scalar_mul(ctx_sc[:], ctx_fin[:], zr_bc[:])
        nc.sync.dma_start(t["ctx_out"][pb].rearrange("(k p) -> p k", p=P),
                          ctx_sc[:])

    # --- software-pipelined main loop over chunk pairs -----------------
    # Per pair q, slot m: hp[m] DR matmul group, then scheduled work:
    #   slot 0: e-mm(q-1, m=6); prefetch et8(q+1)
    #   slot 1: e-mm(q-1, m=7, stop); [pair 0: z block + deferred tanh]
    #   slot 2: exp+bcast A(q-1); e-mm(q, 0)
    #   slot 3: exp+bcast B(q-1); amr A(q-1); e-mm(q, 1)
    #   slot 4: amr B(q-1); e-mm(q, 2)
    #   slot 5: e-mm(q, 3); prefetch etb(q+1)
    #   slot 6+: e-mm(q, m-2)
    prev = None          # state of pair q-1: dict with e-mm/post leftovers
    for q in range(NQ):
        b, sp = divmod(q, NPAIR)
        ss0, ss1 = 2 * sp, 2 * sp + 1
        if sp == 0:
            state[b] = {
                "z_parts": spool.tile([1, NSS], F32, name="z_parts",
                                      tag="z_parts"),
                "creds": [],
            }
        et0 = et8_tiles.pop((b, ss0))
        et1 = et8_tiles.pop((b, ss1))
        e_ps0 = e_ps1 = None
        hq = {}          # hp PSUM tiles awaiting tanh (pair-0 deferral)
        hs = {}          # h SBUF tiles by m

        def emit_tanh(m, hp0, hp1):
            h0 = hpool.tile([P, S_SUB], BF16, name="h_sb", tag="h_sb")
            nc.scalar.activation(h0[:], hp0[:], AFT.Tanh,
                                 bias=z_sb[m][:, b:b + 1],
                                 scale=1.0 / W1_SCALE)
            h1 = hpool.tile([P, S_SUB], BF16, name="h_sb", tag="h_sb")
            nc.scalar.activation(h1[:], hp1[:], AFT.Tanh,
                                 bias=z_sb[m][:, b:b + 1],
                                 scale=1.0 / W1_SCALE)
            hs[m] = (h0, h1)

        def emit_escore(me):
            hh0, hh1 = hs.pop(me)
            nc.tensor.matmul(e_ps0[:], lhsT=w2_all[:, me, :], rhs=hh0[:],
                             start=(me == 0), stop=(me == KC - 1))
            nc.tensor.matmul(e_ps1[:], lhsT=w2_all[:, me, :], rhs=hh1[:],
                             start=(me == 0), stop=(me == KC - 1))

        for m in range(KC):
            # hp matmul group for both chunks, weights shared per kp
            hp0 = ppa.tile([P, S_SUB], F32, name="hp", tag="ppa_t")
            hp1 = ppa.tile([P, S_SUB], F32, name="hp", tag="ppa_t")
            for kp in range(KP):
                lw = w1e_all[:, 2 * kp:2 * kp + 2, m * P:(m + 1) * P]
                nc.tensor.matmul(hp0[:], lhsT=lw,
                                 rhs=et0[:, 2 * kp:2 * kp + 2, :],
                                 start=(kp == 0), stop=(kp == KP - 1),
                                 perf_mode=DR)
                nc.tensor.matmul(hp1[:], lhsT=lw,
                                 rhs=et1[:, 2 * kp:2 * kp + 2, :],
                                 start=(kp == 0), stop=(kp == KP - 1),
                                 perf_mode=DR)
            # scheduled non-hp work for this slot
            if m == 0:
                if prev is not None:
                    prev["escore"](6)
                if q + 1 < NQ:
                    dma_pair8(q + 1)
            elif m == 1:
                if prev is not None:
                    prev["escore"](7)
            elif m == 2:
                if prev is not None:
                    prev["p_bc0"] = emit_exp(prev["pend0"])
                e_ps0 = ppe.tile([2, S_SUB], F32, name="e_ps", tag="ppe_t")
                e_ps1 = ppe.tile([2, S_SUB], F32, name="e_ps", tag="ppe_t")
                emit_escore(0)
            elif m == 3:
                if prev is not None:
                    prev["p_bc1"] = emit_exp(prev["pend1"])
                    emit_ctx(prev["pend0"], prev["p_bc0"])
                emit_escore(1)
            elif m == 4:
                if prev is not None:
                    emit_ctx(prev["pend1"], prev["p_bc1"])
                    prev = None
                emit_escore(2)
            elif m == 5:
                emit_escore(3)
                if q + 1 < NQ:
                    dma_pairb(q + 1)
            else:
                emit_escore(m - 2)
            # tanh for this slot's hp (deferred on pair 0 until z exists)
            if q == 0 and m < 2:
                hq[m] = (hp0, hp1)
                if m == 1:
                    emit_z_block()
                    emit_tanh(0, *hq.pop(0))
                    emit_tanh(1, *hq.pop(1))
            else:
                emit_tanh(m, hp0, hp1)

        def make_escore(e0, e1, hsd):
            def f(me):
                hh0, hh1 = hsd.pop(me)
                nc.tensor.matmul(e0[:], lhsT=w2_all[:, me, :], rhs=hh0[:],
                                 start=False, stop=(me == KC - 1))
                nc.tensor.matmul(e1[:], lhsT=w2_all[:, me, :], rhs=hh1[:],
                                 start=False, stop=(me == KC - 1))
            return f

        prev = {
            "escore": make_escore(e_ps0, e_ps1, hs),
            "pend0": (b, ss0, e_ps0),
            "pend1": (b, ss1, e_ps1),
        }

    # drain the last pair
    prev["escore"](6)
    prev["escore"](7)
    p0 = emit_exp(prev["pend0"])
    emit_ctx(prev["pend0"], p0)
    p1 = emit_exp(prev["pend1"])
    emit_ctx(prev["pend1"], p1)
    state.clear()


def build_program():
    nc = bacc.Bacc("TRN2", target_bir_lowering=False, debug=False,
                   num_devices=NCORES)
    t = declare_io(nc, input_kind="ExternalInput")
    with tile.TileContext(nc) as tc:
        with ExitStack() as ctx:
            _body(ctx, tc, t)
    nc.compile()
    return nc


def prep_in_maps(inputs):
    f8 = ml_dtypes.float8_e4m3
    bf = ml_dtypes.bfloat16
    enc = np.asarray(inputs["encoder_outputs"], dtype=np.float32)
    hid = np.asarray(inputs["hidden_state"], dtype=np.float32)
    W1 = np.asarray(inputs["W1"], dtype=np.float32)
    b1 = np.asarray(inputs["b1"], dtype=np.float32)
    w2 = np.asarray(inputs["w2"], dtype=np.float32)
    encT = enc.transpose(0, 2, 1)                 # [B, D, S] strided view
    encT8 = encT.astype(f8)
    encTb = encT.astype(bf)
    w1e8 = (W1[:D] * np.float32(W1_SCALE)).astype(f8)
    w1hb = W1[D:].astype(bf)
    b1c = np.ascontiguousarray(b1.reshape(D, 1))
    w2c = np.zeros((D, 2), dtype=bf)
    w2c[:, 0] = w2.astype(bf)
    in_maps = []
    for c in range(NCORES):
        sl = slice(c * BL, (c + 1) * BL)
        in_maps.append({
            "encT8": encT8[sl],
            "encTb": encTb[sl],
            "hidT": np.ascontiguousarray(hid[sl].T).astype(bf),
            "w1e8": w1e8,
            "w1h": w1hb,
            "b1": b1c,
            "w2": w2c,
        })
    return in_maps


_NC_CACHE = None


def kernel(**inputs):
    global _NC_CACHE
    if _NC_CACHE is None:
        _NC_CACHE = build_program()
    nc = _NC_CACHE
    in_maps = prep_in_maps(inputs)
    res = run_bass_kernel_spmd(nc, in_maps, core_ids=list(range(NCORES)))
    out = np.empty((B, D), dtype=np.float32)
    for c in range(NCORES):
        out[c * BL:(c + 1) * BL] = res.results[c]["ctx_out"]
    return out


# revision 18
# speedup vs baseline: 1.1081x; 1.1081x over previous
"""Bahdanau-attention kernel for Trainium2, 8-core data-parallel over batch.

Problem: context = softmax(w2 . tanh(enc @ W1_enc + hid @ W1_hid + b1)) @ enc
  B=32, S=2048, D=1024.  Each of the 8 cores handles 4 batch elements.

Device-side strategy (per core, per batch b, per 512-wide seq chunk):
  - the heavy enc @ W1_enc matmul runs in fp8-e4m3 with DoubleRow perf
    mode: both operands quantized to e4m3 (W1 pre-scaled by 64 into the
    e4m3 normal range), two 128-deep k-tiles contracted per PE pass.
    The 1/64 de-scale rides the ACT tanh's per-op scale for free.
  - chunks are processed in PAIRS sharing each stationary weight load:
    LDWEIGHTS costs ~470ns when weights change between matmuls but is
    fully hidden when two consecutive matmuls share them (measured).
  - tanh+bias via ACT: h = tanh(hp/64 + z[m]), output bf16.
  - e-scores: e_row[2, 512] += w2[m]^T @ h[m] in bf16 on the PE,
    emitted TWO m-steps behind the hp matmuls (m=6,7 spill into the
    next pair's first two slots) so the PE never waits on ACT.
  - softmax post-work runs entirely off the PE: p_row = exp(e_row) on
    ACT (fused accum_out -> normalizer partial; no max subtraction:
    |e| <= sum|w2| ~ 26, safe fp32), p broadcast to 128 partitions on
    GPSIMD, context partials on DVE via fused affine_mul_reduce over a
    separate bf16 copy of encT (one op per k-group does multiply by p
    and the 512-wide reduction).
  - z = hid @ W1_hid + b1 computed as hid-stationary matmuls (tiny
    4-col weight loads) giving zT[4, 1024], then PE-transposed per
    m-chunk via a 4x4 identity and bias-added on DVE.  Emitted inside
    pair 0's slot 1 so the PE never waits on the W1_hid DMA.
  - context[b] = ctx * (1/Z), 1/Z partition-broadcast on GPSIMD.

Quantization error budget (validated vs the fp32 reference on the
harness inputs, and on hardware): fp8 enc/W1 + bf16 h/w2/hid/W1h ==>
rel_err ~1.22e-2 against the 2e-2 gate.
"""

import numpy as np
import ml_dtypes
from contextlib import ExitStack

import concourse.bacc as bacc
import concourse.tile as tile
from concourse import mybir
from concourse.bass_utils import run_bass_kernel_spmd

AFT = mybir.ActivationFunctionType
ALU = mybir.AluOpType
F32 = mybir.dt.float32
BF16 = mybir.dt.bfloat16
F8 = mybir.dt.float8e4
DR = mybir.MatmulPerfMode.DoubleRow

B, S, D = 32, 2048, 1024
NCORES = 8
BL = B // NCORES          # 4 batch elements per core
P = 128
KC = D // P               # 8 contraction / output chunks
KP = KC // 2              # 4 DoubleRow k-pairs
S_SUB = 512               # seq chunk processed per inner iteration
NSS = S // S_SUB          # 4
NPAIR = NSS // 2          # chunk pairs per batch
NQ = BL * NPAIR           # total pairs
W1_SCALE = 64.0           # fp8 pre-scale on W1_enc (power of two)


def declare_io(nc, input_kind="ExternalInput"):
    t = {}
    t["encT8"] = nc.dram_tensor("encT8", [BL, D, S], F8, kind=input_kind).ap()
    t["encTb"] = nc.dram_tensor("encTb", [BL, D, S], BF16,
                                kind=input_kind).ap()
    t["hidT"] = nc.dram_tensor("hidT", [D, BL], BF16, kind=input_kind).ap()
    t["w1e8"] = nc.dram_tensor("w1e8", [D, D], F8, kind=input_kind).ap()
    t["w1h"] = nc.dram_tensor("w1h", [D, D], BF16, kind=input_kind).ap()
    t["b1"] = nc.dram_tensor("b1", [D, 1], F32, kind=input_kind).ap()
    t["w2"] = nc.dram_tensor("w2", [D, 2], BF16, kind=input_kind).ap()
    t["ctx_out"] = nc.dram_tensor("ctx_out", [BL, D], F32,
                                  kind="ExternalOutput").ap()
    return t


def _body(ctx, tc, t):
    nc = tc.nc
    id4 = nc.inline_tensor(np.eye(BL, dtype=np.float32), name="id4").ap()

    const = ctx.enter_context(tc.tile_pool(name="const", bufs=1))
    wpool = ctx.enter_context(tc.tile_pool(name="wpool", bufs=1))
    epool = ctx.enter_context(tc.tile_pool(name="epool", bufs=4))
    bpool = ctx.enter_context(tc.tile_pool(name="bpool", bufs=6))
    hpool = ctx.enter_context(tc.tile_pool(name="hpool", bufs=8))
    spool = ctx.enter_context(tc.tile_pool(name="spool", bufs=2))
    pbpool = ctx.enter_context(tc.tile_pool(name="pbpool", bufs=2))
    s2pool = ctx.enter_context(tc.tile_pool(name="s2pool", bufs=2))
    cpool = ctx.enter_context(tc.tile_pool(name="cpool", bufs=2 * NSS + 2))
    # PSUM budget (8 banks): hp 4 + (e_ps | ztp) 2 + zT halves 2
    ppa = ctx.enter_context(tc.tile_pool(name="ppa", bufs=4, space="PSUM"))
    ppe = ctx.enter_context(tc.tile_pool(name="ppe", bufs=2, space="PSUM"))
    pzt = ctx.enter_context(tc.tile_pool(name="pzt", bufs=2, space="PSUM"))

    encT8, encTb = t["encT8"], t["encTb"]

    # --- phase 0 DMAs, one instruction per tensor (HWDGE costs ~625ns
    # of serialized issue per DMA instruction, so batch via 3D access
    # patterns).  Order = first-use order on the PE.
    w1e_all = wpool.tile([P, KC, D], F8, name="w1e_all")
    nc.sync.dma_start(w1e_all[:],
                      t["w1e8"].rearrange("(k p) m -> p k m", p=P))

    et8_tiles, etb_tiles = {}, {}
    encT8r = [encT8[b].rearrange("(k p) s -> p k s", p=P)
              for b in range(BL)]
    encTbr = [encTb[b].rearrange("(k p) s -> p k s", p=P)
              for b in range(BL)]

    def dma_pair8(q):
        """fp8 et for both chunks of pair q (feeds the hp matmuls)."""
        b, sp = divmod(q, NPAIR)
        for ss in (2 * sp, 2 * sp + 1):
            e8 = epool.tile([P, KC, S_SUB], F8, name="et8", tag="et8")
            nc.sync.dma_start(
                e8[:], encT8r[b][:, :, ss * S_SUB:(ss + 1) * S_SUB])
            et8_tiles[(b, ss)] = e8

    def dma_pairb(q):
        """bf16 et for both chunks of pair q (feeds the DVE context)."""
        b, sp = divmod(q, NPAIR)
        for ss in (2 * sp, 2 * sp + 1):
            eb = bpool.tile([P, KC, S_SUB], BF16, name="etb", tag="etb")
            nc.sync.dma_start(
                eb[:], encTbr[b][:, :, ss * S_SUB:(ss + 1) * S_SUB])
            etb_tiles[(b, ss)] = eb

    dma_pair8(0)

    hid_all = const.tile([P, KC, BL], BF16, name="hid_all")
    nc.sync.dma_start(hid_all[:],
                      t["hidT"].rearrange("(k p) b -> p k b", p=P))
    w1h_all = wpool.tile([P, KC, D], BF16, name="w1h_all")
    nc.sync.dma_start(w1h_all[:],
                      t["w1h"].rearrange("(k p) m -> p k m", p=P))
    b1_all = const.tile([P, KC], F32, name="b1_all")
    nc.sync.dma_start(b1_all[:],
                      t["b1"].rearrange("(k p) o -> p (k o)", p=P))
    w2_all = const.tile([P, KC, 2], BF16, name="w2_all")
    nc.sync.dma_start(w2_all[:],
                      t["w2"].rearrange("(k p) c -> p k c", p=P))
    id4_t = const.tile([BL, BL], F32, name="id4_t")
    nc.sync.dma_start(id4_t[:], id4)
    dma_pairb(0)

    z_sb = []            # [P, BL] per m, filled by emit_z_block

    def emit_z_block():
        """z = hid @ W1_hid + b1 via hid-stationary matmuls (zT[4, e])
        + per-m PE transpose.  Sits in pair 0's slot 1 so the W1_hid
        DMA has landed by the time the PE gets here."""
        zt0 = pzt.tile([BL, S_SUB], F32, name="zt0", tag="zt")
        zt1 = pzt.tile([BL, S_SUB], F32, name="zt1", tag="zt")
        for k in range(KC):
            nc.tensor.matmul(zt0[:], lhsT=hid_all[:, k, :],
                             rhs=w1h_all[:, k, 0:S_SUB],
                             start=(k == 0), stop=(k == KC - 1))
            nc.tensor.matmul(zt1[:], lhsT=hid_all[:, k, :],
                             rhs=w1h_all[:, k, S_SUB:D],
                             start=(k == 0), stop=(k == KC - 1))
        zts = const.tile([BL, D], F32, name="zts")
        nc.vector.tensor_copy(zts[:, 0:S_SUB], zt0[:])
        nc.vector.tensor_copy(zts[:, S_SUB:D], zt1[:])
        for m in range(KC):
            ztp = ppe.tile([P, BL], F32, name="ztp", tag="ppe_t")
            nc.tensor.transpose(ztp[:], zts[:, m * P:(m + 1) * P], id4_t[:])
            zt = const.tile([P, BL], F32, name=f"z_{m}")
            nc.vector.tensor_scalar_add(zt[:], ztp[:], b1_all[:, m:m + 1])
            z_sb.append(zt)

    state = {}           # per-batch: z_parts tile + per-chunk cred tiles

    def emit_exp(pend):
        """exp + gpsimd partition-broadcast for a finished chunk; returns
        the broadcast tile for the amr stage."""
        pb, pss, e_ps = pend
        st = state[pb]
        p_row = spool.tile([1, S_SUB], F32, name="p_row", tag="p_row")
        nc.scalar.activation(p_row[:], e_ps[0:1, :], AFT.Exp,
                             accum_out=st["z_parts"][0:1, pss:pss + 1])
        p_bc = pbpool.tile([P, S_SUB], F32, name="p_bc", tag="p_bc")
        nc.gpsimd.partition_broadcast(p_bc[:], p_row[:])
        return p_bc

    def emit_ctx(pend, p_bc):
        """DVE context partials for a finished chunk (+finalize)."""
        pb, pss, _ = pend
        st = state[pb]
        etb = etb_tiles.pop((pb, pss))
        cred = cpool.tile([P, KC], F32, name="cred", tag="cred")
        for k in range(KC):
            scr = s2pool.tile([P, S_SUB], BF16, name="scr", tag="scr")
            nc.vector.affine_mul_reduce(
                out=scr[:], accum_out=cred[:, k:k + 1],
                in0=etb[:, k, :], in1=p_bc[:], scale=1.0, bias=0.0)
        st["creds"].append(cred)
        if pss == NSS - 1:
            emit_finalize(pb)

    def emit_finalize(pb):
        """context[pb] = (sum of chunk contributions) / Z, then store."""
        st = state.pop(pb)
        zsum = spool.tile([1, 1], F32, name="zsum", tag="zsum")
        nc.vector.tensor_reduce(zsum[:], st["z_parts"][:],
                                axis=mybir.AxisListType.X, op=ALU.add)
        zr = spool.tile([1, 1], F32, name="zr", tag="zr")
        nc.vector.reciprocal(zr[:], zsum[:])
        zr_bc = spool.tile([P, 1], F32, name="zr_bc", tag="zr_bc")
        nc.gpsimd.partition_broadcast(zr_bc[:], zr[:])
        creds = st["creds"]
        ctx_fin = cpool.tile([P, KC], F32, name="ctx_fin", tag="ctx_fin")
        nc.vector.tensor_tensor(ctx_fin[:], creds[0][:], creds[1][:], ALU.add)
        nc.vector.tensor_tensor(ctx_fin[:], ctx_fin[:], creds[2][:], ALU.add)
        nc.vector.tensor_tensor(ctx_fin[:], ctx_fin[:], creds[3][:], ALU.add)
        ctx_sc = cpool.tile([P, KC], F32, name="ctx_sc", tag="ctx_sc")
        nc.vector.tensor_scalar_mul(ctx_sc[:], ctx_fin[:], zr_bc[:])
        nc.sync.dma_start(t["ctx_out"][pb].rearrange("(k p) -> p k", p=P),
                          ctx_sc[:])

    # --- software-pipelined main loop over chunk pairs -----------------
    # Per pair q, slot m: hp[m] DR matmul group, then scheduled work:
    #   slot 0: e-mm(q-1, m=6); prefetch et8(q+1)
    #   slot 1: e-mm(q-1, m=7, stop); [pair 0: z block + deferred tanh]
    #   slot 2: exp+bcast A(q-1); e-mm(q, 0)
    #   slot 3: exp+bcast B(q-1); amr A(q-1); e-mm(q, 1)
    #   slot 4: amr B(q-1); e-mm(q, 2)
    #   slot 5: e-mm(q, 3); prefetch etb(q+1)
    #   slot 6+: e-mm(q, m-2)
    prev = None          # state of pair q-1: dict with e-mm/post leftovers
    for q in range(NQ):
        b, sp = divmod(q, NPAIR)
        ss0, ss1 = 2 * sp, 2 * sp + 1
        if sp == 0:
            state[b] = {
                "z_parts": spool.tile([1, NSS], F32, name="z_parts",
                                      tag="z_parts"),
                "creds": [],
            }
        et0 = et8_tiles.pop((b, ss0))
        et1 = et8_tiles.pop((b, ss1))
        e_ps0 = e_ps1 = None
        hq = {}          # hp PSUM tiles awaiting tanh (pair-0 deferral)
        hs = {}          # h SBUF tiles by m

        def emit_tanh(m, hp0, hp1):
            h0 = hpool.tile([P, S_SUB], BF16, name="h_sb", tag="h_sb")
            nc.scalar.activation(h0[:], hp0[:], AFT.Tanh,
                                 bias=z_sb[m][:, b:b + 1],
                                 scale=1.0 / W1_SCALE)
            h1 = hpool.tile([P, S_SUB], BF16, name="h_sb", tag="h_sb")
            nc.scalar.activation(h1[:], hp1[:], AFT.Tanh,
                                 bias=z_sb[m][:, b:b + 1],
                                 scale=1.0 / W1_SCALE)
            hs[m] = (h0, h1)

        def emit_escore(me):
            hh0, hh1 = hs.pop(me)
            nc.tensor.matmul(e_ps0[:], lhsT=w2_all[:, me, :], rhs=hh0[:],
                             start=(me == 0), stop=(me == KC - 1))
            nc.tensor.matmul(e_ps1[:], lhsT=w2_all[:, me, :], rhs=hh1[:],
                             start=(me == 0), stop=(me == KC - 1))

        for m in range(KC):
            # hp matmul group for both chunks, weights shared per kp
            hp0 = ppa.tile([P, S_SUB], F32, name="hp", tag="ppa_t")
            hp1 = ppa.tile([P, S_SUB], F32, name="hp", tag="ppa_t")
            for kp in range(KP):
                lw = w1e_all[:, 2 * kp:2 * kp + 2, m * P:(m + 1) * P]
                nc.tensor.matmul(hp0[:], lhsT=lw,
                                 rhs=et0[:, 2 * kp:2 * kp + 2, :],
                                 start=(kp == 0), stop=(kp == KP - 1),
                                 perf_mode=DR)
                nc.tensor.matmul(hp1[:], lhsT=lw,
                                 rhs=et1[:, 2 * kp:2 * kp + 2, :],
                                 start=(kp == 0), stop=(kp == KP - 1),
                                 perf_mode=DR)
            # scheduled non-hp work for this slot
            if m == 0:
                if prev is not None:
                    prev["escore"](6)
                if q + 1 < NQ:
                    dma_pair8(q + 1)
            elif m == 1:
                if prev is not None:
                    prev["escore"](7)
            elif m == 2:
                if prev is not None:
                    prev["p_bc0"] = emit_exp(prev["pend0"])
                e_ps0 = ppe.tile([2, S_SUB], F32, name="e_ps", tag="ppe_t")
                e_ps1 = ppe.tile([2, S_SUB], F32, name="e_ps", tag="ppe_t")
                emit_escore(0)
            elif m == 3:
                if prev is not None:
                    prev["p_bc1"] = emit_exp(prev["pend1"])
                    emit_ctx(prev["pend0"], prev["p_bc0"])
                emit_escore(1)
            elif m == 4:
                if prev is not None:
                    emit_ctx(prev["pend1"], prev["p_bc1"])
                    prev = None
                emit_escore(2)
            elif m == 5:
                emit_escore(3)
                if q + 1 < NQ:
                    dma_pairb(q + 1)
            else:
                emit_escore(m - 2)
            # tanh for this slot's hp (deferred on pair 0 until z exists)
            if q == 0 and m < 2:
                hq[m] = (hp0, hp1)
                if m == 1:
                    emit_z_block()
                    emit_tanh(0, *hq.pop(0))
                    emit_tanh(1, *hq.pop(1))
            else:
                emit_tanh(m, hp0, hp1)

        def make_escore(e0, e1, hsd):
            def f(me):
                hh0, hh1 = hsd.pop(me)
                nc.tensor.matmul(e0[:], lhsT=w2_all[:, me, :], rhs=hh0[:],
                                 start=False, stop=(me == KC - 1))
                nc.tensor.matmul(e1[:], lhsT=w2_all[:, me, :], rhs=hh1[:],
                                 start=False, stop=(me == KC - 1))
            return f

        prev = {
            "escore": make_escore(e_ps0, e_ps1, hs),
            "pend0": (b, ss0, e_ps0),
            "pend1": (b, ss1, e_ps1),
        }

    # drain the last pair
    prev["escore"](6)
    prev["escore"](7)
    p0 = emit_exp(prev["pend0"])
    emit_ctx(prev["pend0"], p0)
    p1 = emit_exp(prev["pend1"])
    emit_ctx(prev["pend1"], p1)
    state.clear()


def build_program():
    nc = bacc.Bacc("TRN2", target_bir_lowering=False, debug=False,
                   num_devices=NCORES)
    t = declare_io(nc, input_kind="ExternalInput")
    with tile.TileContext(nc) as tc:
        with ExitStack() as ctx:
            _body(ctx, tc, t)
    nc.compile()
    return nc


def prep_in_maps(inputs):
    f8 = ml_dtypes.float8_e4m3
    bf = ml_dtypes.bfloat16
    enc = np.asarray(inputs["encoder_outputs"], dtype=np.float32)
    hid = np.asarray(inputs["hidden_state"], dtype=np.float32)
    W1 = np.asarray(inputs["W1"], dtype=np.float32)
    b1 = np.asarray(inputs["b1"], dtype=np.float32)
    w2 = np.asarray(inputs["w2"], dtype=np.float32)
    encT = enc.transpose(0, 2, 1)                 # [B, D, S] strided view
    encT8 = encT.astype(f8)
    encTb = encT.astype(bf)
    w1e8 = (W1[:D] * np.float32(W1_SCALE)).astype(f8)
    w1hb = W1[D:].astype(bf)
    b1c = np.ascontiguousarray(b1.reshape(D, 1))
    w2c = np.zeros((D, 2), dtype=bf)
    w2c[:, 0] = w2.astype(bf)
    in_maps = []
    for c in range(NCORES):
        sl = slice(c * BL, (c + 1) * BL)
        in_maps.append({
            "encT8": encT8[sl],
            "encTb": encTb[sl],
            "hidT": np.ascontiguousarray(hid[sl].T).astype(bf),
            "w1e8": w1e8,
            "w1h": w1hb,
            "b1": b1c,
            "w2": w2c,
        })
    return in_maps


_NC_CACHE = None


def kernel(**inputs):
    global _NC_CACHE
    if _NC_CACHE is None:
        _NC_CACHE = build_program()
    nc = _NC_CACHE
    in_maps = prep_in_maps(inputs)
    res = run_bass_kernel_spmd(nc, in_maps, core_ids=list(range(NCORES)))
    out = np.empty((B, D), dtype=np.float32)
    for c in range(NCORES):
        out[c * BL:(c + 1) * BL] = res.results[c]["ctx_out"]
    return out


# revision 21
# speedup vs baseline: 1.3118x; 1.1839x over previous
"""Bahdanau-attention kernel for Trainium2, 8-core data-parallel over batch.

Problem: context = softmax(w2 . tanh(enc @ W1_enc + hid @ W1_hid + b1)) @ enc
  B=32, S=2048, D=1024.  Each of the 8 cores handles 4 batch elements.

Device-side strategy (per core, per batch b, per 512-wide seq chunk):
  - the heavy enc @ W1_enc matmul runs in fp8-e4m3 with DoubleRow perf
    mode: both operands quantized to e4m3 (W1 pre-scaled by 64 into the
    e4m3 normal range), two 128-deep k-tiles contracted per PE pass.
    The 1/64 de-scale rides the ACT tanh's per-op scale for free.
  - chunks are processed in PAIRS sharing each stationary weight load:
    LDWEIGHTS costs ~470ns when weights change between matmuls but is
    fully hidden when two consecutive matmuls share them (measured).
  - tanh+bias via ACT: h = tanh(hp/64 + z[m]), output bf16.
  - e-scores: e_row[2, 512] += w2[m]^T @ h[m] in bf16 on the PE,
    emitted TWO m-steps behind the hp matmuls (m=6,7 spill into the
    next pair's first two slots) so the PE never waits on ACT.
  - softmax post-work runs entirely off the PE: p_row = exp(e_row) on
    ACT (fused accum_out -> normalizer partial; no max subtraction:
    |e| <= sum|w2| ~ 26, safe fp32), p broadcast to 128 partitions on
    GPSIMD, context partials on DVE via fused affine_mul_reduce over a
    separate bf16 copy of encT (one op per k-group does multiply by p
    and the 512-wide reduction).
  - z = hid @ W1_hid + b1 computed as hid-stationary matmuls (tiny
    4-col weight loads) giving zT[4, 1024], then PE-transposed per
    m-chunk via a 4x4 identity and bias-added on DVE.  Emitted inside
    pair 0's slot 1 so the PE never waits on the W1_hid DMA.
  - context[b] = ctx * (1/Z), 1/Z partition-broadcast on GPSIMD.

Quantization error budget (validated vs the fp32 reference on the
harness inputs, and on hardware): fp8 enc/W1 + bf16 h/w2/hid/W1h ==>
rel_err ~1.22e-2 against the 2e-2 gate.
"""

import numpy as np
import ml_dtypes
from contextlib import ExitStack

import concourse.bacc as bacc
import concourse.tile as tile
from concourse import mybir
from concourse.bass_utils import run_bass_kernel_spmd

AFT = mybir.ActivationFunctionType
ALU = mybir.AluOpType
F32 = mybir.dt.float32
BF16 = mybir.dt.bfloat16
F8 = mybir.dt.float8e4
DR = mybir.MatmulPerfMode.DoubleRow

B, S, D = 32, 2048, 1024
NCORES = 8
BL = B // NCORES          # 4 batch elements per core
P = 128
KC = D // P               # 8 contraction / output chunks
KP = KC // 2              # 4 DoubleRow k-pairs
S_SUB = 512               # seq chunk processed per inner iteration
NSS = S // S_SUB          # 4
NPAIR = NSS // 2          # chunk pairs per batch
NQ = BL * NPAIR           # total pairs
W1_SCALE = 64.0           # fp8 pre-scale on W1_enc (power of two)


def declare_io(nc, input_kind="ExternalInput"):
    t = {}
    t["encT8"] = nc.dram_tensor("encT8", [BL, D, S], F8, kind=input_kind).ap()
    t["encTb"] = nc.dram_tensor("encTb", [BL, D, S], BF16,
                                kind=input_kind).ap()
    t["hidT"] = nc.dram_tensor("hidT", [D, BL], BF16, kind=input_kind).ap()
    t["w1e8"] = nc.dram_tensor("w1e8", [D, D], F8, kind=input_kind).ap()
    t["w1h"] = nc.dram_tensor("w1h", [D, D], BF16, kind=input_kind).ap()
    t["b1"] = nc.dram_tensor("b1", [D, 1], F32, kind=input_kind).ap()
    t["w2"] = nc.dram_tensor("w2", [D, 2], BF16, kind=input_kind).ap()
    t["ctx_out"] = nc.dram_tensor("ctx_out", [BL, D], F32,
                                  kind="ExternalOutput").ap()
    return t


def _body(ctx, tc, t):
    nc = tc.nc
    id4 = nc.inline_tensor(np.eye(BL, dtype=np.float32), name="id4").ap()

    const = ctx.enter_context(tc.tile_pool(name="const", bufs=1))
    wpool = ctx.enter_context(tc.tile_pool(name="wpool", bufs=1))
    epool = ctx.enter_context(tc.tile_pool(name="epool", bufs=4))
    bpool = ctx.enter_context(tc.tile_pool(name="bpool", bufs=6))
    hpool = ctx.enter_context(tc.tile_pool(name="hpool", bufs=8))
    spool = ctx.enter_context(tc.tile_pool(name="spool", bufs=2))
    pbpool = ctx.enter_context(tc.tile_pool(name="pbpool", bufs=2))
    s2pool = ctx.enter_context(tc.tile_pool(name="s2pool", bufs=2))
    cpool = ctx.enter_context(tc.tile_pool(name="cpool", bufs=2 * NSS + 2))
    # PSUM budget (8 banks): hp 4 + (e_ps | ztp) 2 + zT halves 2
    ppa = ctx.enter_context(tc.tile_pool(name="ppa", bufs=4, space="PSUM"))
    ppe = ctx.enter_context(tc.tile_pool(name="ppe", bufs=2, space="PSUM"))
    pzt = ctx.enter_context(tc.tile_pool(name="pzt", bufs=2, space="PSUM"))

    encT8, encTb = t["encT8"], t["encTb"]

    # --- phase 0 DMAs, one instruction per tensor (HWDGE costs ~625ns
    # of serialized issue per DMA instruction, so batch via 3D access
    # patterns).  Order = first-use order on the PE.
    w1e_all = wpool.tile([P, KC, D], F8, name="w1e_all")
    nc.sync.dma_start(w1e_all[:],
                      t["w1e8"].rearrange("(k p) m -> p k m", p=P))

    et8_tiles, etb_tiles = {}, {}
    encT8r = [encT8[b].rearrange("(k p) s -> p k s", p=P)
              for b in range(BL)]
    encTbr = [encTb[b].rearrange("(k p) s -> p k s", p=P)
              for b in range(BL)]

    def dma_pair8(q):
        """fp8 et for both chunks of pair q (feeds the hp matmuls)."""
        b, sp = divmod(q, NPAIR)
        for ss in (2 * sp, 2 * sp + 1):
            e8 = epool.tile([P, KC, S_SUB], F8, name="et8", tag="et8")
            nc.sync.dma_start(
                e8[:], encT8r[b][:, :, ss * S_SUB:(ss + 1) * S_SUB])
            et8_tiles[(b, ss)] = e8

    def dma_pairb(q):
        """bf16 et for both chunks of pair q (feeds the DVE context)."""
        b, sp = divmod(q, NPAIR)
        for ss in (2 * sp, 2 * sp + 1):
            eb = bpool.tile([P, KC, S_SUB], BF16, name="etb", tag="etb")
            nc.sync.dma_start(
                eb[:], encTbr[b][:, :, ss * S_SUB:(ss + 1) * S_SUB])
            etb_tiles[(b, ss)] = eb

    dma_pair8(0)

    hid_all = const.tile([P, KC, BL], BF16, name="hid_all")
    nc.sync.dma_start(hid_all[:],
                      t["hidT"].rearrange("(k p) b -> p k b", p=P))
    w1h_all = wpool.tile([P, KC, D], BF16, name="w1h_all")
    nc.sync.dma_start(w1h_all[:],
                      t["w1h"].rearrange("(k p) m -> p k m", p=P))
    b1_all = const.tile([P, KC], F32, name="b1_all")
    nc.sync.dma_start(b1_all[:],
                      t["b1"].rearrange("(k p) o -> p (k o)", p=P))
    w2_all = const.tile([P, KC, 2], BF16, name="w2_all")
    nc.sync.dma_start(w2_all[:],
                      t["w2"].rearrange("(k p) c -> p k c", p=P))
    id4_t = const.tile([BL, BL], F32, name="id4_t")
    nc.sync.dma_start(id4_t[:], id4)
    dma_pairb(0)

    z_sb = []            # [P, BL] per m, filled by emit_z_block

    def emit_z_block():
        """z = hid @ W1_hid + b1 via hid-stationary matmuls (zT[4, e])
        + per-m PE transpose.  Sits in pair 0's slot 1 so the W1_hid
        DMA has landed by the time the PE gets here."""
        zt0 = pzt.tile([BL, S_SUB], F32, name="zt0", tag="zt")
        zt1 = pzt.tile([BL, S_SUB], F32, name="zt1", tag="zt")
        for k in range(KC):
            nc.tensor.matmul(zt0[:], lhsT=hid_all[:, k, :],
                             rhs=w1h_all[:, k, 0:S_SUB],
                             start=(k == 0), stop=(k == KC - 1))
            nc.tensor.matmul(zt1[:], lhsT=hid_all[:, k, :],
                             rhs=w1h_all[:, k, S_SUB:D],
                             start=(k == 0), stop=(k == KC - 1))
        zts = const.tile([BL, D], F32, name="zts")
        nc.vector.tensor_copy(zts[:, 0:S_SUB], zt0[:])
        nc.vector.tensor_copy(zts[:, S_SUB:D], zt1[:])
        for m in range(KC):
            ztp = ppe.tile([P, BL], F32, name="ztp", tag="ppe_t")
            nc.tensor.transpose(ztp[:], zts[:, m * P:(m + 1) * P], id4_t[:])
            zt = const.tile([P, BL], F32, name=f"z_{m}")
            nc.vector.tensor_scalar_add(zt[:], ztp[:], b1_all[:, m:m + 1])
            z_sb.append(zt)

    state = {}           # per-batch: z_parts tile + per-chunk cred tiles

    def emit_exp(pend):
        """exp + gpsimd partition-broadcast for a finished chunk; returns
        the broadcast tile for the amr stage."""
        pb, pss, e_ps = pend
        st = state[pb]
        p_row = spool.tile([1, S_SUB], F32, name="p_row", tag="p_row")
        nc.scalar.activation(p_row[:], e_ps[0:1, :], AFT.Exp,
                             accum_out=st["z_parts"][0:1, pss:pss + 1])
        p_bc = pbpool.tile([P, S_SUB], F32, name="p_bc", tag="p_bc")
        nc.gpsimd.partition_broadcast(p_bc[:], p_row[:])
        return p_bc

    def emit_ctx(pend, p_bc):
        """DVE context partials for a finished chunk (+finalize)."""
        pb, pss, _ = pend
        st = state[pb]
        etb = etb_tiles.pop((pb, pss))
        cred = cpool.tile([P, KC], F32, name="cred", tag="cred")
        for k in range(KC):
            scr = s2pool.tile([P, S_SUB], BF16, name="scr", tag="scr")
            nc.vector.affine_mul_reduce(
                out=scr[:], accum_out=cred[:, k:k + 1],
                in0=etb[:, k, :], in1=p_bc[:], scale=1.0, bias=0.0)
        st["creds"].append(cred)
        if pss == NSS - 1:
            emit_finalize(pb)

    def emit_finalize(pb):
        """context[pb] = (sum of chunk contributions) / Z, then store."""
        st = state.pop(pb)
        zsum = spool.tile([1, 1], F32, name="zsum", tag="zsum")
        nc.vector.tensor_reduce(zsum[:], st["z_parts"][:],
                                axis=mybir.AxisListType.X, op=ALU.add)
        zr = spool.tile([1, 1], F32, name="zr", tag="zr")
        nc.vector.reciprocal(zr[:], zsum[:])
        zr_bc = spool.tile([P, 1], F32, name="zr_bc", tag="zr_bc")
        nc.gpsimd.partition_broadcast(zr_bc[:], zr[:])
        creds = st["creds"]
        ctx_fin = cpool.tile([P, KC], F32, name="ctx_fin", tag="ctx_fin")
        nc.vector.tensor_tensor(ctx_fin[:], creds[0][:], creds[1][:], ALU.add)
        nc.vector.tensor_tensor(ctx_fin[:], ctx_fin[:], creds[2][:], ALU.add)
        nc.vector.tensor_tensor(ctx_fin[:], ctx_fin[:], creds[3][:], ALU.add)
        ctx_sc = cpool.tile([P, KC], F32, name="ctx_sc", tag="ctx_sc")
        nc.vector.tensor_scalar_mul(ctx_sc[:], ctx_fin[:], zr_bc[:])
        nc.sync.dma_start(t["ctx_out"][pb].rearrange("(k p) -> p k", p=P),
                          ctx_sc[:])

    # --- software-pipelined main loop over chunk pairs -----------------
    # Per pair q, slot m: hp[m] DR matmul group, then scheduled work:
    #   slot 0: e-mm(q-1, m=6); prefetch et8(q+1)
    #   slot 1: e-mm(q-1, m=7, stop); [pair 0: z block + deferred tanh]
    #   slot 2: exp+bcast A(q-1); e-mm(q, 0)
    #   slot 3: exp+bcast B(q-1); amr A(q-1); e-mm(q, 1)
    #   slot 4: amr B(q-1); e-mm(q, 2)
    #   slot 5: e-mm(q, 3); prefetch etb(q+1)
    #   slot 6+: e-mm(q, m-2)
    prev = None          # state of pair q-1: dict with e-mm/post leftovers
    for q in range(NQ):
        b, sp = divmod(q, NPAIR)
        ss0, ss1 = 2 * sp, 2 * sp + 1
        if sp == 0:
            state[b] = {
                "z_parts": spool.tile([1, NSS], F32, name="z_parts",
                                      tag="z_parts"),
                "creds": [],
            }
        et0 = et8_tiles.pop((b, ss0))
        et1 = et8_tiles.pop((b, ss1))
        e_ps0 = e_ps1 = None
        hq = {}          # hp PSUM tiles awaiting tanh (pair-0 deferral)
        hs = {}          # h SBUF tiles by m

        def emit_tanh(m, hp0, hp1):
            h0 = hpool.tile([P, S_SUB], BF16, name="h_sb", tag="h_sb")
            nc.scalar.activation(h0[:], hp0[:], AFT.Tanh,
                                 bias=z_sb[m][:, b:b + 1],
                                 scale=1.0 / W1_SCALE)
            h1 = hpool.tile([P, S_SUB], BF16, name="h_sb", tag="h_sb")
            nc.scalar.activation(h1[:], hp1[:], AFT.Tanh,
                                 bias=z_sb[m][:, b:b + 1],
                                 scale=1.0 / W1_SCALE)
            hs[m] = (h0, h1)

        def emit_escore(me):
            hh0, hh1 = hs.pop(me)
            nc.tensor.matmul(e_ps0[:], lhsT=w2_all[:, me, :], rhs=hh0[:],
                             start=(me == 0), stop=(me == KC - 1))
            nc.tensor.matmul(e_ps1[:], lhsT=w2_all[:, me, :], rhs=hh1[:],
                             start=(me == 0), stop=(me == KC - 1))

        for m in range(KC):
            # hp matmul group for both chunks, weights shared per kp
            hp0 = ppa.tile([P, S_SUB], F32, name="hp", tag="ppa_t")
            hp1 = ppa.tile([P, S_SUB], F32, name="hp", tag="ppa_t")
            for kp in range(KP):
                lw = w1e_all[:, 2 * kp:2 * kp + 2, m * P:(m + 1) * P]
                nc.tensor.matmul(hp0[:], lhsT=lw,
                                 rhs=et0[:, 2 * kp:2 * kp + 2, :],
                                 start=(kp == 0), stop=(kp == KP - 1),
                                 perf_mode=DR)
                nc.tensor.matmul(hp1[:], lhsT=lw,
                                 rhs=et1[:, 2 * kp:2 * kp + 2, :],
                                 start=(kp == 0), stop=(kp == KP - 1),
                                 perf_mode=DR)
            # scheduled non-hp work for this slot
            if m == 0:
                if prev is not None:
                    prev["escore"](6)
                if q + 1 < NQ:
                    dma_pair8(q + 1)
            elif m == 1:
                if prev is not None:
                    prev["escore"](7)
            elif m == 2:
                if prev is not None:
                    prev["p_bc0"] = emit_exp(prev["pend0"])
                e_ps0 = ppe.tile([2, S_SUB], F32, name="e_ps", tag="ppe_t")
                e_ps1 = ppe.tile([2, S_SUB], F32, name="e_ps", tag="ppe_t")
                emit_escore(0)
            elif m == 3:
                if prev is not None:
                    prev["p_bc1"] = emit_exp(prev["pend1"])
                    emit_ctx(prev["pend0"], prev["p_bc0"])
                emit_escore(1)
            elif m == 4:
                if prev is not None:
                    emit_ctx(prev["pend1"], prev["p_bc1"])
                    prev = None
                emit_escore(2)
            elif m == 5:
                emit_escore(3)
                if q + 1 < NQ:
                    dma_pairb(q + 1)
            else:
                emit_escore(m - 2)
            # tanh for this slot's hp (deferred on pair 0 until z exists)
            if q == 0 and m < 2:
                hq[m] = (hp0, hp1)
                if m == 1:
                    emit_z_block()
                    emit_tanh(0, *hq.pop(0))
                    emit_tanh(1, *hq.pop(1))
            else:
                emit_tanh(m, hp0, hp1)

        def make_escore(e0, e1, hsd):
            def f(me):
                hh0, hh1 = hsd.pop(me)
                nc.tensor.matmul(e0[:], lhsT=w2_all[:, me, :], rhs=hh0[:],
                                 start=False, stop=(me == KC - 1))
                nc.tensor.matmul(e1[:], lhsT=w2_all[:, me, :], rhs=hh1[:],
                                 start=False, stop=(me == KC - 1))
            return f

        prev = {
            "escore": make_escore(e_ps0, e_ps1, hs),
            "pend0": (b, ss0, e_ps0),
            "pend1": (b, ss1, e_ps1),
        }

    # drain the last pair
    prev["escore"](6)
    prev["escore"](7)
    p0 = emit_exp(prev["pend0"])
    emit_ctx(prev["pend0"], p0)
    p1 = emit_exp(prev["pend1"])
    emit_ctx(prev["pend1"], p1)
    state.clear()


def build_program():
    nc = bacc.Bacc("TRN2", target_bir_lowering=False, debug=False,
                   num_devices=NCORES)
    t = declare_io(nc, input_kind="ExternalInput")
    with tile.TileContext(nc) as tc:
        with ExitStack() as ctx:
            _body(ctx, tc, t)
    nc.compile()
    return nc


def prep_in_maps(inputs):
    f8 = ml_dtypes.float8_e4m3
    bf = ml_dtypes.bfloat16
    enc = np.asarray(inputs["encoder_outputs"], dtype=np.float32)
    hid = np.asarray(inputs["hidden_state"], dtype=np.float32)
    W1 = np.asarray(inputs["W1"], dtype=np.float32)
    b1 = np.asarray(inputs["b1"], dtype=np.float32)
    w2 = np.asarray(inputs["w2"], dtype=np.float32)
    encT = enc.transpose(0, 2, 1)                 # [B, D, S] strided view
    encT8 = encT.astype(f8)
    encTb = encT.astype(bf)
    w1e8 = (W1[:D] * np.float32(W1_SCALE)).astype(f8)
    w1hb = W1[D:].astype(bf)
    b1c = np.ascontiguousarray(b1.reshape(D, 1))
    w2c = np.zeros((D, 2), dtype=bf)
    w2c[:, 0] = w2.astype(bf)
    in_maps = []
    for c in range(NCORES):
        sl = slice(c * BL, (c + 1) * BL)
        in_maps.append({
            "encT8": encT8[sl],
            "encTb": encTb[sl],
            "hidT": np.ascontiguousarray(hid[sl].T).astype(bf),
            "w1e8": w1e8,
            "w1h": w1hb,
            "b1": b1c,
            "w2": w2c,
        })
    return in_maps


_NC_CACHE = None


def kernel(**inputs):
    global _NC_CACHE
    if _NC_CACHE is None:
        _NC_CACHE = build_program()
    nc = _NC_CACHE
    in_maps = prep_in_maps(inputs)
    res = run_bass_kernel_spmd(nc, in_maps, core_ids=list(range(NCORES)))
    out = np.empty((B, D), dtype=np.float32)
    for c in range(NCORES):
        out[c * BL:(c + 1) * BL] = res.results[c]["ctx_out"]
    return out
